# revision 19
# baseline (speedup 1.0000x reference)
"""Trainium2 Bass kernel for CoarseMatching (dual-softmax feature matching).

Computes, for inputs f1, f2 of shape [N=4, L=4800, C=256]:
    sim  = (f1*s) @ (f2*s)^T / T          (s = C^-0.5, T = 0.1)
    conf = softmax(sim, axis=1) * softmax(sim, axis=2)
plus thresholding / mutual-nearest-neighbour outputs.

Sharding: data-parallel over batch N (4 batches x 2 cores); within a batch
element the L rows are split in half across the 2 cores.  Each core runs a
single fused kernel with two phases.  Phase 1 (transposed orientation)
computes this shard's partial column sums of exp(sim); a pair-wise device
AllReduce (cores {2n, 2n+1}) completes the column-softmax denominator — the
cross-L-shard reduction from the sharding hint.  Phase 2 recomputes exp(sim)
in row orientation and normalizes to conf = E^2/(rowsum*colsum) on device.
The cheap O(L)/bool derived outputs (row/col max, mask, argmax) are formed on
the host from the device conf exactly as the reference does.

Matmul precision strategy: the tensor engine's float32r mode runs at full
rate (1 cyc/row vs 4 for fp32) but consumes only 10 explicit mantissa bits.
Phase 1 runs plain float32r on host-pre-rounded inputs: its output only
feeds 4800-term column sums, where per-element input-rounding noise averages
down to ~1e-5.  Phase 2 (whose matmul error hits conf directly) uses a
3-term hi/lo split (hi.hi + hi.lo + lo.hi, each term exact in float32r)
giving ~2^-21 effective input precision at 3/4 the PE cost of fp32.
"""

import sys

if "/opt/trn_rl_repo" not in sys.path:
    sys.path.insert(0, "/opt/trn_rl_repo")

import os as _os

import numpy as np

N, L, C = 4, 4800, 256
S = L
HALF = L // 2
N_CORES = 8
TEMPERATURE = 0.1
CONFIDENCE_THRESHOLD = 0.2
INV_TEMP = 1.0 / TEMPERATURE  # exp scale applied on device
FEAT_SCALE = np.float32(1.0 / (C**0.5))

# "fused" = single launch with device AllReduce; "twopass" = two launches
# with the 19KB colsum reduction done on host between them.
KERNEL_MODE = _os.environ.get("KERNEL_MODE", "fused")
PASS1_MM_DTYPE = _os.environ.get("KERNEL_P1_DTYPE", "float32r")
# "float32" (exact, 4 cyc/row) or "split3" (3x float32r hi/lo, ~fp32 quality)
PASS2_MODE = _os.environ.get("KERNEL_P2_MODE", "split3")

_BUILD_CACHE = {}

# perf info (exec_time_ns etc.) from the most recent kernel() call, one entry
# per device launch; populated when tracing is enabled (BASS_TRACE=1)
LAST_PERF = []


def _geometry(F):
    """Split free dim F into PSUM blocks (<= 1536 f32 = 3 banks) of matmul
    chunks.  Chunk starts are bank-aligned (512 multiples) and widths are
    256..512 so float32r runs at full rate; block starts are multiples of 128
    so phase-1 partition strips never straddle per-block input tiles."""
    if F == 4800:
        return [
            (0, [512, 512, 512]),
            (1536, [512, 512, 512]),
            (3072, [512, 512, 256]),
            (4352, [448]),
        ]
    if F == 2400:
        return [(0, [512, 512, 512]), (1536, [512, 352])]
    # generic fallback (used by small simulator tests)
    out, f0 = [], 0
    while f0 < F:
        bw = min(1536, F - f0)
        cws, c = [], 0
        while c < bw:
            cw = min(512, bw - c)
            cws.append(cw)
            c += cw
        out.append((f0, cws))
        f0 += bw
    return out


def round_mantissa(x, keep_bits=10):
    """Round fp32 mantissa to keep_bits explicit bits (RNE) — the precision
    the fp32r matmul mode actually consumes; pre-rounding makes it exact."""
    xi = x.view(np.uint32).astype(np.uint64)
    drop = 23 - keep_bits
    half = np.uint64(1 << (drop - 1))
    one = np.uint64(1)
    lsb_mask = np.uint64((1 << drop) - 1)
    rounded = (xi + half - one + ((xi >> np.uint64(drop)) & one)) & ~lsb_mask
    return rounded.astype(np.uint32).view(np.float32)


def split_hi_lo(x, keep_bits=10):
    """x -> (hi, lo): hi = RNE-rounded to keep_bits mantissa bits, lo = the
    exact fp32 residual rounded to keep_bits bits."""
    hi = round_mantissa(x, keep_bits)
    lo = round_mantissa((x - hi).astype(np.float32), keep_bits)
    return hi, lo


class _P:
    """Per-build context bag."""


def _load_blocked(nc, consts, param, F, mm_dt, name):
    """DMA a [128, 2, F] DRAM param into per-geometry-block SBUF tiles so the
    first matmuls only wait for the first block's load.  Returns a lookup
    f(k, a, b) -> AP covering columns [a, b) (must lie inside one block)."""
    blocks = _geometry(F)
    tiles = []
    for f0, cws in blocks:
        bw = sum(cws)
        t = consts.tile([128, 2, bw], mm_dt, name=f"{name}_b{f0}")
        nc.sync.dma_start(out=t[:], in_=param[:, :, f0 : f0 + bw])
        tiles.append((f0, bw, t))

    def lookup(k, a, b):
        for f0, bw, t in tiles:
            if a >= f0 and b <= f0 + bw:
                return t[:, k, a - f0 : b - f0]
        raise AssertionError(f"range [{a},{b}) straddles block tiles")

    return lookup


def _emit_pass1(nc, tc, pools, f1_lookup, f2_lookup, cp_sb, P, F, mybir):
    """Transposed orientation: for each 128-wide column strip of s, compute
    colpart[s] = sum_l exp(sim[s, l]) over this core's l rows."""
    F32 = mybir.dt.float32
    AF = mybir.ActivationFunctionType
    AX = mybir.AxisListType
    psum, etmp, small = pools["psum"], pools["etmp"], pools["small"]
    blocks = _geometry(F)
    nstrip = (P + 127) // 128
    for j in range(nstrip):
        p0 = j * 128
        pl = min(128, P - p0)
        parts = small.tile([128, len(blocks)], F32, tag="parts1", name=f"parts1_{j}")
        for bi, (f0, cws) in enumerate(blocks):
            bw = sum(cws)
            ps = psum.tile([128, 1536], F32, tag="ps", name=f"ps1_{j}_{bi}")
            c0 = 0
            for cw in cws:
                for k in range(2):
                    nc.tensor.matmul(
                        ps[:pl, c0 : c0 + cw],
                        lhsT=f2_lookup(k, p0, p0 + pl),
                        rhs=f1_lookup(k, f0 + c0, f0 + c0 + cw),
                        start=(k == 0),
                        stop=(k == 1),
                    )
                c0 += cw
            e = etmp.tile([128, 1536], F32, tag="e", name=f"e1_{j}_{bi}")
            nc.scalar.activation(
                out=e[:pl, :bw],
                in_=ps[:pl, :bw],
                func=AF.Exp,
                scale=float(INV_TEMP),
                accum_out=parts[:pl, bi : bi + 1],
            )
        nc.vector.reduce_sum(out=cp_sb[:pl, j : j + 1], in_=parts[:pl, :], axis=AX.X)


def _emit_pass2(nc, tc, pools, mm_pair_lookups, rc_sb, conf_out, P, F, mybir):
    """Row orientation: for each 128-row strip of l, compute
    conf = ((E * 1/rowsum) * E) * (1/colsum broadcast) and DMA it out.
    mm_pair_lookups: list of (f1_lookup, f2_lookup) accumulation terms."""
    F32 = mybir.dt.float32
    AF = mybir.ActivationFunctionType
    AX = mybir.AxisListType
    ALU = mybir.AluOpType
    psum, strip_pool, small = pools["psum"], pools["strip"], pools["small"]
    blocks = _geometry(F)
    nstrip = (P + 127) // 128
    nterm = len(mm_pair_lookups)
    for i in range(nstrip):
        p0 = i * 128
        pl = min(128, P - p0)
        parts = small.tile([128, len(blocks)], F32, tag="parts2", name=f"parts2_{i}")
        E = strip_pool.tile([128, F], F32, tag="E", name=f"E_{i}")
        for bi, (f0, cws) in enumerate(blocks):
            bw = sum(cws)
            ps = psum.tile([128, 1536], F32, tag="ps", name=f"ps2_{i}_{bi}")
            c0 = 0
            for cw in cws:
                nmm = 0
                for f1_lookup, f2_lookup in mm_pair_lookups:
                    for k in range(2):
                        nc.tensor.matmul(
                            ps[:pl, c0 : c0 + cw],
                            lhsT=f1_lookup(k, p0, p0 + pl),
                            rhs=f2_lookup(k, f0 + c0, f0 + c0 + cw),
                            start=(nmm == 0),
                            stop=(nmm == 2 * nterm - 1),
                        )
                        nmm += 1
                c0 += cw
            nc.scalar.activation(
                out=E[:pl, f0 : f0 + bw],
                in_=ps[:pl, :bw],
                func=AF.Exp,
                scale=float(INV_TEMP),
                accum_out=parts[:pl, bi : bi + 1],
            )
        rowsum = small.tile([128, 1], F32, tag="rowsum", name=f"rowsum_{i}")
        rr = small.tile([128, 1], F32, tag="rr", name=f"rr_{i}")
        nc.vector.reduce_sum(out=rowsum[:pl], in_=parts[:pl, :], axis=AX.X)
        nc.vector.reciprocal(out=rr[:pl], in_=rowsum[:pl])
        # conf in place in E (SBUF budget: no room for a second strip tile
        # alongside the four hi/lo operand tensors)
        nc.vector.scalar_tensor_tensor(
            out=E[:pl],
            in0=E[:pl],
            scalar=rr[:pl],
            in1=E[:pl],
            op0=ALU.mult,
            op1=ALU.mult,
        )
        # column-scale multiply: 1/3 of strips on DVE, 2/3 on GPSIMD (2x
        # slower there) so both engines finish together
        eng = nc.vector if (i % 3) == 0 else nc.gpsimd
        eng.tensor_tensor(out=E[:pl], in0=E[:pl], in1=rc_sb[:pl], op=ALU.mult)
        nc.sync.dma_start(out=conf_out[p0 : p0 + pl, :], in_=E[:pl, :])


def build_fused_nc(P_rows=HALF, F_cols=S, replica_groups=None):
    """Single-launch fused kernel: phase 1 colsum partials, pair AllReduce,
    reciprocal + broadcast, phase 2 conf.  Inputs: f1hi/f1lo [128,2,P_rows],
    f2hi/f2lo [128,2,F_cols] (float32r hi/lo split); output conf [P_rows,
    F_cols] f32."""
    import concourse.bacc as bacc
    import concourse.tile as tile
    from concourse import mybir
    from contextlib import ExitStack

    F32 = mybir.dt.float32
    mm_dt = mybir.dt.float32r
    if replica_groups is None:
        replica_groups = [[0, 1], [2, 3], [4, 5], [6, 7]]

    nc = bacc.Bacc(None, target_bir_lowering=False, num_devices=N_CORES)
    f1hi = nc.declare_dram_parameter("f1hi", [128, 2, P_rows], mm_dt, isOutput=False)
    f1lo = nc.declare_dram_parameter("f1lo", [128, 2, P_rows], mm_dt, isOutput=False)
    f2hi = nc.declare_dram_parameter("f2hi", [128, 2, F_cols], mm_dt, isOutput=False)
    f2lo = nc.declare_dram_parameter("f2lo", [128, 2, F_cols], mm_dt, isOutput=False)
    conf_out = nc.declare_dram_parameter("conf", [P_rows, F_cols], F32, isOutput=True)
    nstrip_s = (F_cols + 127) // 128
    npad = nstrip_s * 128

    with ExitStack() as ctx:
        tc = ctx.enter_context(tile.TileContext(nc))
        consts = ctx.enter_context(tc.tile_pool(name="consts", bufs=1))
        pools = {
            "psum": ctx.enter_context(tc.tile_pool(name="psum", bufs=2, space="PSUM")),
            "etmp": ctx.enter_context(tc.tile_pool(name="etmp", bufs=2)),
            "strip": ctx.enter_context(tc.tile_pool(name="strip", bufs=2)),
            "small": ctx.enter_context(tc.tile_pool(name="small", bufs=4)),
        }
        dram = ctx.enter_context(tc.tile_pool(name="dram", bufs=1, space="DRAM"))

        # phase-1-critical loads first: f1hi (rhs, full rows) + f2hi blocks
        # (lhsT strips); lo tensors only gate phase 2.
        f1hi_sb = consts.tile([128, 2, P_rows], mm_dt)
        nc.sync.dma_start(out=f1hi_sb[:], in_=f1hi[:])
        f2hi_lookup = _load_blocked(nc, consts, f2hi, F_cols, mm_dt, "f2hi")
        f1lo_sb = consts.tile([128, 2, P_rows], mm_dt)
        nc.sync.dma_start(out=f1lo_sb[:], in_=f1lo[:])
        f2lo_lookup = _load_blocked(nc, consts, f2lo, F_cols, mm_dt, "f2lo")

        def f1hi_lookup(k, a, b):
            return f1hi_sb[:, k, a:b]

        def f1lo_lookup(k, a, b):
            return f1lo_sb[:, k, a:b]

        cp_sb = consts.tile([128, nstrip_s], F32)
        nc.vector.memset(cp_sb[:], 0.0)

        _emit_pass1(nc, tc, pools, f1hi_lookup, f2hi_lookup, cp_sb, F_cols, P_rows, mybir)

        # pair AllReduce of colsum partials (19 KB), then 1/colsum broadcast
        cp_local = dram.tile([128, nstrip_s], F32)
        cp_red = dram.tile([128, nstrip_s], F32)
        nc.sync.dma_start(out=cp_local[:], in_=cp_sb[:])
        nc.gpsimd.collective_compute(
            "AllReduce",
            mybir.AluOpType.add,
            replica_groups=replica_groups,
            ins=[cp_local[:]],
            outs=[cp_red[:]],
        )
        cp_red_sb = consts.tile([128, nstrip_s], F32)
        nc.sync.dma_start(out=cp_red_sb[:], in_=cp_red[:])
        rc_small = consts.tile([128, nstrip_s], F32)
        nc.vector.reciprocal(out=rc_small[:], in_=cp_red_sb[:])
        rc_lin = dram.tile([npad], F32)
        nc.sync.dma_start(
            out=rc_lin[:].rearrange("(j p) -> p j", p=128), in_=rc_small[:]
        )
        rc_sb = consts.tile([128, F_cols], F32)
        nc.sync.dma_start(
            out=rc_sb[:], in_=rc_lin[None, :F_cols].to_broadcast((128, F_cols))
        )

        mm_pairs = [
            (f1hi_lookup, f2hi_lookup),
            (f1hi_lookup, f2lo_lookup),
            (f1lo_lookup, f2hi_lookup),
        ]
        _emit_pass2(nc, tc, pools, mm_pairs, rc_sb, conf_out, P_rows, F_cols, mybir)
    nc.compile()
    return nc


def build_colsum_nc(P=S, F=HALF, mm_dtype=PASS1_MM_DTYPE):
    """Two-launch variant, pass 1: per-core partial column sums.
    inputs f1t [128,2,F], f2t [128,2,P]; output colpart [128, nstrip]."""
    import concourse.bacc as bacc
    import concourse.tile as tile
    from concourse import mybir
    from contextlib import ExitStack

    F32 = mybir.dt.float32
    mm_dt = getattr(mybir.dt, mm_dtype)
    nc = bacc.Bacc(None, target_bir_lowering=False)
    f1t = nc.declare_dram_parameter("f1t", [128, 2, F], mm_dt, isOutput=False)
    f2t = nc.declare_dram_parameter("f2t", [128, 2, P], mm_dt, isOutput=False)
    nstrip = (P + 127) // 128
    colpart = nc.declare_dram_parameter("colpart", [128, nstrip], F32, isOutput=True)

    with ExitStack() as ctx:
        tc = ctx.enter_context(tile.TileContext(nc))
        consts = ctx.enter_context(tc.tile_pool(name="consts", bufs=1))
        pools = {
            "psum": ctx.enter_context(tc.tile_pool(name="psum", bufs=2, space="PSUM")),
            "etmp": ctx.enter_context(tc.tile_pool(name="etmp", bufs=3)),
            "small": ctx.enter_context(tc.tile_pool(name="small", bufs=4)),
        }
        f1t_sb = consts.tile([128, 2, F], mm_dt)
        nc.sync.dma_start(out=f1t_sb[:], in_=f1t[:])
        f2t_lookup = _load_blocked(nc, consts, f2t, P, mm_dt, "f2t")
        cp_sb = consts.tile([128, nstrip], F32)
        nc.vector.memset(cp_sb[:], 0.0)

        def f1t_lookup(k, a, b):
            return f1t_sb[:, k, a:b]

        _emit_pass1(nc, tc, pools, f1t_lookup, f2t_lookup, cp_sb, P, F, mybir)
        nc.sync.dma_start(out=colpart[:], in_=cp_sb[:])
    nc.compile()
    return nc


def build_conf_nc(P=HALF, F=S, mode=PASS2_MODE):
    """Two-launch variant, pass 2: conf rows for this core's row shard.
    mode "float32": inputs f1t/f2t fp32; "split3": f1hi/f1lo/f2hi/f2lo
    float32r.  Plus rcol [F] = 1/colsum; output conf [P, F]."""
    import concourse.bacc as bacc
    import concourse.tile as tile
    from concourse import mybir
    from contextlib import ExitStack

    F32 = mybir.dt.float32
    nc = bacc.Bacc(None, target_bir_lowering=False)
    names = (
        ["f1hi", "f1lo", "f2hi", "f2lo"] if mode == "split3" else ["f1t", "f2t"]
    )
    mm_dt = mybir.dt.float32r if mode == "split3" else mybir.dt.float32
    params = {}
    for nm in names:
        dim = P if nm.startswith("f1") else F
        params[nm] = nc.declare_dram_parameter(nm, [128, 2, dim], mm_dt, isOutput=False)
    rcol = nc.declare_dram_parameter("rcol", [F], F32, isOutput=False)
    conf_out = nc.declare_dram_parameter("conf", [P, F], F32, isOutput=True)

    with ExitStack() as ctx:
        tc = ctx.enter_context(tile.TileContext(nc))
        consts = ctx.enter_context(tc.tile_pool(name="consts", bufs=1))
        pools = {
            "psum": ctx.enter_context(tc.tile_pool(name="psum", bufs=2, space="PSUM")),
            "strip": ctx.enter_context(tc.tile_pool(name="strip", bufs=2)),
            "small": ctx.enter_context(tc.tile_pool(name="small", bufs=4)),
        }
        lookups = {}
        for nm in names:
            if nm.startswith("f1"):
                t = consts.tile([128, 2, P], mm_dt, name=f"{nm}_sb")
                nc.sync.dma_start(out=t[:], in_=params[nm][:])
                lookups[nm] = (lambda t: lambda k, a, b: t[:, k, a:b])(t)
            else:
                lookups[nm] = _load_blocked(nc, consts, params[nm], F, mm_dt, nm)
        rc_sb = consts.tile([128, F], F32)
        nc.sync.dma_start(out=rc_sb[:], in_=rcol[None, :].to_broadcast((128, F)))

        if mode == "split3":
            mm_pairs = [
                (lookups["f1hi"], lookups["f2hi"]),
                (lookups["f1hi"], lookups["f2lo"]),
                (lookups["f1lo"], lookups["f2hi"]),
            ]
        else:
            mm_pairs = [(lookups["f1t"], lookups["f2t"])]
        _emit_pass2(nc, tc, pools, mm_pairs, rc_sb, conf_out, P, F, mybir)
    nc.compile()
    return nc


def _to_kmajor(x):
    """[Rows, C] f32 -> [128, 2, Rows] with (p, k) = (c % 128, c // 128)."""
    return np.ascontiguousarray(x.T.reshape(2, 128, -1).transpose(1, 0, 2))


_HARDENED = False


def _harden_tracing():
    """Make trace=True (BASS_TRACE=1) survivable in this container: the image's
    antenv lacks axon_hooks (NTFF hook module), and artifact upload has no
    egress. Without this, enabling tracing crashes run_bass_kernel_spmd."""
    global _HARDENED
    if _HARDENED:
        return
    _HARDENED = True
    import types
    import contextlib
    import ctypes

    try:
        import antenv.axon_hooks  # noqa: F401
    except ImportError:
        mod = types.ModuleType("antenv.axon_hooks")
        holder = {"hook": None}
        mod.set_axon_ntff_profile_hook = lambda h: holder.__setitem__("hook", h)
        mod.get_axon_ntff_profile_hook = lambda: holder["hook"]
        try:
            import antenv

            antenv.axon_hooks = mod
        except ImportError:
            pass
        sys.modules["antenv.axon_hooks"] = mod
        try:
            lib = ctypes.CDLL("/opt/axon/libaxon_pjrt.so")
            if hasattr(lib, "axon_start_nrt_profile"):
                lib.axon_start_nrt_profile.argtypes = [
                    ctypes.POINTER(ctypes.c_int64),
                    ctypes.c_size_t,
                ]
                lib.axon_start_nrt_profile.restype = ctypes.c_int64
                lib.axon_stop_nrt_profile.argtypes = [ctypes.c_char_p]
                lib.axon_stop_nrt_profile.restype = ctypes.c_int64

                @contextlib.contextmanager
                def _hook(output_dir, device_ids):
                    import jax

                    jax.devices()
                    if device_ids:
                        ids = (ctypes.c_int64 * len(device_ids))(*device_ids)
                        rc = lib.axon_start_nrt_profile(ids, len(device_ids))
                    else:
                        rc = lib.axon_start_nrt_profile(None, 0)
                    if rc != 0:
                        raise RuntimeError(f"axon_start_nrt_profile rc={rc}")
                    try:
                        yield
                    finally:
                        n = lib.axon_stop_nrt_profile(str(output_dir).encode())
                        print(f"ntff profile: {n} file(s) -> {output_dir}")

                mod.set_axon_ntff_profile_hook(_hook)
        except OSError:
            pass

    from concourse import bass_utils as _bu

    if not getattr(_bu.upload_artifacts, "_is_safe_wrapper", False):
        _orig = _bu.upload_artifacts

        def _safe_upload(tmpdir):
            try:
                return _orig(tmpdir)
            except Exception:
                return str(tmpdir)

        _safe_upload._is_safe_wrapper = True
        _bu.upload_artifacts = _safe_upload


def _host_outputs(conf):
    """Derive the four cheap outputs from conf exactly as the reference."""
    row_max = conf.max(axis=2, keepdims=True)
    col_max = conf.max(axis=1, keepdims=True)
    match_mask = (
        (conf > np.float32(CONFIDENCE_THRESHOLD)) & (conf == row_max) & (conf == col_max)
    )
    column_indices = np.argmax(match_mask, axis=2).astype(np.int32)
    valid = np.any(match_mask, axis=2)
    mc = np.take_along_axis(conf, column_indices[..., None], axis=2)[..., 0]
    matching_confidences = np.where(valid, mc, np.float32(0.0)).astype(np.float32)
    return (matching_confidences, valid, column_indices, match_mask, conf)


def _kernel_fused(f1t_per_core, f2t_per_batch):
    from concourse.bass_utils import run_bass_kernel_spmd

    key = ("fused",)
    if key not in _BUILD_CACHE:
        _BUILD_CACHE[key] = build_fused_nc()
    nc = _BUILD_CACHE[key]

    f1_hl = [split_hi_lo(a) for a in f1t_per_core]
    f2_hl = [split_hi_lo(a) for a in f2t_per_batch]
    in_maps = [
        {
            "f1hi": f1_hl[k][0],
            "f1lo": f1_hl[k][1],
            "f2hi": f2_hl[k // 2][0],
            "f2lo": f2_hl[k // 2][1],
        }
        for k in range(N_CORES)
    ]
    res = run_bass_kernel_spmd(nc, in_maps, core_ids=list(range(N_CORES)))
    LAST_PERF.append(("fused", res.exec_time_ns, res.mean_exec_time_ns))
    conf = np.empty((N, L, S), dtype=np.float32)
    for k in range(N_CORES):
        n, h = k // 2, k % 2
        conf[n, h * HALF : (h + 1) * HALF, :] = res.results[k]["conf"]
    return conf


def _kernel_twopass(f1t_per_core, f2t_per_batch):
    from concourse.bass_utils import run_bass_kernel_spmd

    key = ("twopass", PASS1_MM_DTYPE, PASS2_MODE)
    if key not in _BUILD_CACHE:
        _BUILD_CACHE[key] = (
            build_colsum_nc(mm_dtype=PASS1_MM_DTYPE),
            build_conf_nc(mode=PASS2_MODE),
        )
    nc1, nc2 = _BUILD_CACHE[key]

    if PASS1_MM_DTYPE == "float32r":
        f2t_p1 = [round_mantissa(a) for a in f2t_per_batch]
        f1t_p1 = [round_mantissa(a) for a in f1t_per_core]
    else:
        f2t_p1, f1t_p1 = f2t_per_batch, f1t_per_core
    in_maps1 = [{"f1t": f1t_p1[k], "f2t": f2t_p1[k // 2]} for k in range(N_CORES)]
    res1 = run_bass_kernel_spmd(nc1, in_maps1, core_ids=list(range(N_CORES)))
    LAST_PERF.append(("colsum", res1.exec_time_ns, res1.mean_exec_time_ns))

    # host all-reduce of the column-sum partials (the L-shard reduction)
    colsum = []
    for n in range(N):
        parts = []
        for k in (2 * n, 2 * n + 1):
            a = res1.results[k]["colpart"]  # [128, nstrip]
            parts.append(a.T.reshape(-1)[:S])
        colsum.append(parts[0] + parts[1])
    rcol = [(1.0 / cs.astype(np.float64)).astype(np.float32) for cs in colsum]

    if PASS2_MODE == "split3":
        f1_hl = [split_hi_lo(a) for a in f1t_per_core]
        f2_hl = [split_hi_lo(a) for a in f2t_per_batch]
        in_maps2 = [
            {
                "f1hi": f1_hl[k][0],
                "f1lo": f1_hl[k][1],
                "f2hi": f2_hl[k // 2][0],
                "f2lo": f2_hl[k // 2][1],
                "rcol": rcol[k // 2],
            }
            for k in range(N_CORES)
        ]
    else:
        in_maps2 = [
            {
                "f1t": f1t_per_core[k],
                "f2t": f2t_per_batch[k // 2],
                "rcol": rcol[k // 2],
            }
            for k in range(N_CORES)
        ]
    res2 = run_bass_kernel_spmd(nc2, in_maps2, core_ids=list(range(N_CORES)))
    LAST_PERF.append(("conf", res2.exec_time_ns, res2.mean_exec_time_ns))

    conf = np.empty((N, L, S), dtype=np.float32)
    for k in range(N_CORES):
        n, h = k // 2, k % 2
        conf[n, h * HALF : (h + 1) * HALF, :] = res2.results[k]["conf"]
    return conf


def kernel(coarse_image_feature_1, coarse_image_feature_2):
    _harden_tracing()

    f1 = np.asarray(coarse_image_feature_1, dtype=np.float32)
    f2 = np.asarray(coarse_image_feature_2, dtype=np.float32)
    f1s = f1 * FEAT_SCALE
    f2s = f2 * FEAT_SCALE

    # per-core inputs: core k -> batch k//2, row half k%2
    f2t_per_batch = [_to_kmajor(f2s[n]) for n in range(N)]
    f1t_per_core = [
        _to_kmajor(f1s[k // 2, (k % 2) * HALF : (k % 2 + 1) * HALF])
        for k in range(N_CORES)
    ]

    LAST_PERF.clear()
    if KERNEL_MODE == "fused":
        conf = _kernel_fused(f1t_per_core, f2t_per_batch)
    else:
        conf = _kernel_twopass(f1t_per_core, f2t_per_batch)
    return _host_outputs(conf)


# revision 22
# speedup vs baseline: 1.0314x; 1.0314x over previous
"""Trainium2 Bass kernel for CoarseMatching (dual-softmax feature matching).

Computes, for inputs f1, f2 of shape [N=4, L=4800, C=256]:
    sim  = (f1*s) @ (f2*s)^T / T          (s = C^-0.5, T = 0.1)
    conf = softmax(sim, axis=1) * softmax(sim, axis=2)
plus thresholding / mutual-nearest-neighbour outputs.

Sharding: data-parallel over batch N (4 batches x 2 cores); within a batch
element the L rows are split in half across the 2 cores.  Each core runs a
single fused kernel with two phases.  Phase 1 (transposed orientation)
computes this shard's partial column sums of exp(sim); a pair-wise device
AllReduce (cores {2n, 2n+1}) completes the column-softmax denominator — the
cross-L-shard reduction from the sharding hint.  Phase 2 recomputes exp(sim)
in row orientation and normalizes to conf = E^2/(rowsum*colsum) on device.
The cheap O(L)/bool derived outputs (row/col max, mask, argmax) are formed on
the host from the device conf exactly as the reference does.

Matmul precision strategy: the tensor engine's float32r mode runs at full
rate (1 cyc/row vs 4 for fp32) but consumes only 10 explicit mantissa bits.
Phase 1 runs plain float32r on host-pre-rounded inputs: its output only
feeds 4800-term column sums, where per-element input-rounding noise averages
down to ~1e-5.  Phase 2 (whose matmul error hits conf directly) uses a
3-term hi/lo split (hi.hi + hi.lo + lo.hi, each term exact in float32r)
giving ~2^-21 effective input precision at 3/4 the PE cost of fp32.
"""

import sys

if "/opt/trn_rl_repo" not in sys.path:
    sys.path.insert(0, "/opt/trn_rl_repo")

import os as _os

import numpy as np

N, L, C = 4, 4800, 256
S = L
HALF = L // 2
N_CORES = 8
TEMPERATURE = 0.1
CONFIDENCE_THRESHOLD = 0.2
INV_TEMP = 1.0 / TEMPERATURE  # exp scale applied on device
FEAT_SCALE = np.float32(1.0 / (C**0.5))

# "fused" = single launch with device AllReduce; "twopass" = two launches
# with the 19KB colsum reduction done on host between them.
KERNEL_MODE = _os.environ.get("KERNEL_MODE", "fused")
PASS1_MM_DTYPE = _os.environ.get("KERNEL_P1_DTYPE", "float32r")
# "float32" (exact, 4 cyc/row) or "split3" (3x float32r hi/lo, ~fp32 quality)
PASS2_MODE = _os.environ.get("KERNEL_P2_MODE", "split3")

_BUILD_CACHE = {}

# perf info (exec_time_ns etc.) from the most recent kernel() call, one entry
# per device launch; populated when tracing is enabled (BASS_TRACE=1)
LAST_PERF = []


def _geometry(F):
    """Split free dim F into PSUM blocks (<= 1536 f32 = 3 banks) of matmul
    chunks.  Chunk starts are bank-aligned (512 multiples) and widths are
    256..512 so float32r runs at full rate; block starts are multiples of 128
    so phase-1 partition strips never straddle per-block input tiles."""
    if F == 4800:
        return [
            (0, [512, 512, 512]),
            (1536, [512, 512, 512]),
            (3072, [512, 512, 256]),
            (4352, [448]),
        ]
    if F == 2400:
        return [(0, [512, 512, 512]), (1536, [512, 352])]
    # generic fallback (used by small simulator tests)
    out, f0 = [], 0
    while f0 < F:
        bw = min(1536, F - f0)
        cws, c = [], 0
        while c < bw:
            cw = min(512, bw - c)
            cws.append(cw)
            c += cw
        out.append((f0, cws))
        f0 += bw
    return out


def round_mantissa(x, keep_bits=10):
    """Round fp32 mantissa to keep_bits explicit bits (RNE) — the precision
    the fp32r matmul mode actually consumes; pre-rounding makes it exact."""
    xi = x.view(np.uint32).astype(np.uint64)
    drop = 23 - keep_bits
    half = np.uint64(1 << (drop - 1))
    one = np.uint64(1)
    lsb_mask = np.uint64((1 << drop) - 1)
    rounded = (xi + half - one + ((xi >> np.uint64(drop)) & one)) & ~lsb_mask
    return rounded.astype(np.uint32).view(np.float32)


def split_hi_lo(x, keep_bits=10):
    """x -> (hi, lo): hi = RNE-rounded to keep_bits mantissa bits, lo = the
    exact fp32 residual rounded to keep_bits bits."""
    hi = round_mantissa(x, keep_bits)
    lo = round_mantissa((x - hi).astype(np.float32), keep_bits)
    return hi, lo


class _P:
    """Per-build context bag."""


def _load_blocked(nc, consts, param, F, mm_dt, name):
    """DMA a [128, 2, F] DRAM param into per-geometry-block SBUF tiles so the
    first matmuls only wait for the first block's load.  Returns a lookup
    f(k, a, b) -> AP covering columns [a, b) (must lie inside one block)."""
    blocks = _geometry(F)
    tiles = []
    for f0, cws in blocks:
        bw = sum(cws)
        t = consts.tile([128, 2, bw], mm_dt, name=f"{name}_b{f0}")
        nc.sync.dma_start(out=t[:], in_=param[:, :, f0 : f0 + bw])
        tiles.append((f0, bw, t))

    def lookup(k, a, b):
        for f0, bw, t in tiles:
            if a >= f0 and b <= f0 + bw:
                return t[:, k, a - f0 : b - f0]
        raise AssertionError(f"range [{a},{b}) straddles block tiles")

    return lookup


def _emit_pass1(nc, tc, pools, f1_lookup, f2_lookup, cp_sb, P, F, mybir,
                strips=None, exp_inplace=False):
    """Transposed orientation: for each 128-wide column strip of s, compute
    colpart[s] = sum_l exp(sim[s, l]) over this core's l rows.
    exp_inplace: write the (unused) exp values back onto the PSUM block
    instead of an SBUF scratch tile (ScE->PSUM is the faster port and only
    accum_out matters)."""
    F32 = mybir.dt.float32
    AF = mybir.ActivationFunctionType
    AX = mybir.AxisListType
    psum, small = pools["psum"], pools["small"]
    etmp = pools.get("etmp")
    blocks = _geometry(F)
    nstrip = (P + 127) // 128
    if strips is None:
        strips = range(nstrip)
    for j in strips:
        p0 = j * 128
        pl = min(128, P - p0)
        parts = small.tile([128, len(blocks)], F32, tag="parts1", name=f"parts1_{j}")
        for bi, (f0, cws) in enumerate(blocks):
            bw = sum(cws)
            ps = psum.tile([128, 1536], F32, tag="ps", name=f"ps1_{j}_{bi}")
            c0 = 0
            for cw in cws:
                for k in range(2):
                    nc.tensor.matmul(
                        ps[:pl, c0 : c0 + cw],
                        lhsT=f2_lookup(k, p0, p0 + pl),
                        rhs=f1_lookup(k, f0 + c0, f0 + c0 + cw),
                        start=(k == 0),
                        stop=(k == 1),
                    )
                c0 += cw
            if exp_inplace:
                eout = ps[:pl, :bw]
            else:
                e = etmp.tile([128, 1536], F32, tag="e", name=f"e1_{j}_{bi}")
                eout = e[:pl, :bw]
            nc.scalar.activation(
                out=eout,
                in_=ps[:pl, :bw],
                func=AF.Exp,
                scale=float(INV_TEMP),
                accum_out=parts[:pl, bi : bi + 1],
            )
        nc.vector.reduce_sum(out=cp_sb[:pl, j : j + 1], in_=parts[:pl, :], axis=AX.X)


def _emit_pass2(nc, tc, pools, mm_pair_lookups, rc_sb, conf_out, P, F, mybir):
    """Row orientation: for each 128-row strip of l, compute
    conf = ((E * 1/rowsum) * E) * (1/colsum broadcast) and DMA it out.
    mm_pair_lookups: list of (f1_lookup, f2_lookup) accumulation terms."""
    F32 = mybir.dt.float32
    AF = mybir.ActivationFunctionType
    AX = mybir.AxisListType
    ALU = mybir.AluOpType
    psum, strip_pool, small = pools["psum"], pools["strip"], pools["small"]
    blocks = _geometry(F)
    nstrip = (P + 127) // 128
    nterm = len(mm_pair_lookups)
    for i in range(nstrip):
        p0 = i * 128
        pl = min(128, P - p0)
        parts = small.tile([128, len(blocks)], F32, tag="parts2", name=f"parts2_{i}")
        E = strip_pool.tile([128, F], F32, tag="E", name=f"E_{i}")
        for bi, (f0, cws) in enumerate(blocks):
            bw = sum(cws)
            ps = psum.tile([128, 1536], F32, tag="ps", name=f"ps2_{i}_{bi}")
            c0 = 0
            for cw in cws:
                nmm = 0
                for f1_lookup, f2_lookup in mm_pair_lookups:
                    for k in range(2):
                        nc.tensor.matmul(
                            ps[:pl, c0 : c0 + cw],
                            lhsT=f1_lookup(k, p0, p0 + pl),
                            rhs=f2_lookup(k, f0 + c0, f0 + c0 + cw),
                            start=(nmm == 0),
                            stop=(nmm == 2 * nterm - 1),
                        )
                        nmm += 1
                c0 += cw
            nc.scalar.activation(
                out=E[:pl, f0 : f0 + bw],
                in_=ps[:pl, :bw],
                func=AF.Exp,
                scale=float(INV_TEMP),
                accum_out=parts[:pl, bi : bi + 1],
            )
        rowsum = small.tile([128, 1], F32, tag="rowsum", name=f"rowsum_{i}")
        rr = small.tile([128, 1], F32, tag="rr", name=f"rr_{i}")
        nc.vector.reduce_sum(out=rowsum[:pl], in_=parts[:pl, :], axis=AX.X)
        nc.vector.reciprocal(out=rr[:pl], in_=rowsum[:pl])
        # conf in place in E (SBUF budget: no room for a second strip tile
        # alongside the four hi/lo operand tensors)
        nc.vector.scalar_tensor_tensor(
            out=E[:pl],
            in0=E[:pl],
            scalar=rr[:pl],
            in1=E[:pl],
            op0=ALU.mult,
            op1=ALU.mult,
        )
        # column-scale multiply: 1/3 of strips on DVE, 2/3 on GPSIMD (2x
        # slower there) so both engines finish together
        eng = nc.vector if (i % 3) == 0 else nc.gpsimd
        eng.tensor_tensor(out=E[:pl], in0=E[:pl], in1=rc_sb[:pl], op=ALU.mult)
        nc.sync.dma_start(out=conf_out[p0 : p0 + pl, :], in_=E[:pl, :])


def build_fused_nc(P_rows=HALF, F_cols=S, replica_groups=None):
    """Single-launch fused kernel: phase 1 colsum partials, pair AllReduce,
    reciprocal + broadcast, phase 2 conf.  Inputs: f1hi/f1lo [128,2,P_rows],
    f2hi/f2lo [128,2,F_cols] (float32r hi/lo split); output conf [P_rows,
    F_cols] f32."""
    import concourse.bacc as bacc
    import concourse.tile as tile
    from concourse import mybir
    from contextlib import ExitStack

    F32 = mybir.dt.float32
    mm_dt = mybir.dt.float32r
    if replica_groups is None:
        replica_groups = [[0, 1], [2, 3], [4, 5], [6, 7]]

    nc = bacc.Bacc(None, target_bir_lowering=False, num_devices=N_CORES)
    f1hi = nc.declare_dram_parameter("f1hi", [128, 2, P_rows], mm_dt, isOutput=False)
    f1lo = nc.declare_dram_parameter("f1lo", [128, 2, P_rows], mm_dt, isOutput=False)
    f2hi = nc.declare_dram_parameter("f2hi", [128, 2, F_cols], mm_dt, isOutput=False)
    f2lo = nc.declare_dram_parameter("f2lo", [128, 2, F_cols], mm_dt, isOutput=False)
    conf_out = nc.declare_dram_parameter("conf", [P_rows, F_cols], F32, isOutput=True)
    nstrip_s = (F_cols + 127) // 128
    npad = nstrip_s * 128

    with ExitStack() as ctx:
        tc = ctx.enter_context(tile.TileContext(nc))
        consts = ctx.enter_context(tc.tile_pool(name="consts", bufs=1))
        pools = {
            "psum": ctx.enter_context(tc.tile_pool(name="psum", bufs=2, space="PSUM")),
            "strip": ctx.enter_context(tc.tile_pool(name="strip", bufs=3)),
            "small": ctx.enter_context(tc.tile_pool(name="small", bufs=4)),
        }
        dram = ctx.enter_context(tc.tile_pool(name="dram", bufs=1, space="DRAM"))

        # phase-1-critical loads first: f1hi (rhs, full rows) + f2hi blocks
        # (lhsT strips); lo tensors only gate phase 2.
        f1hi_sb = consts.tile([128, 2, P_rows], mm_dt)
        nc.sync.dma_start(out=f1hi_sb[:], in_=f1hi[:])
        f2hi_lookup = _load_blocked(nc, consts, f2hi, F_cols, mm_dt, "f2hi")
        f1lo_sb = consts.tile([128, 2, P_rows], mm_dt)
        nc.sync.dma_start(out=f1lo_sb[:], in_=f1lo[:])
        f2lo_lookup = _load_blocked(nc, consts, f2lo, F_cols, mm_dt, "f2lo")

        def f1hi_lookup(k, a, b):
            return f1hi_sb[:, k, a:b]

        def f1lo_lookup(k, a, b):
            return f1lo_sb[:, k, a:b]

        cp_sb = consts.tile([128, nstrip_s], F32)
        nc.vector.memset(cp_sb[:], 0.0)

        # phase 1 in two halves, each followed by its own pair-AllReduce of
        # the colsum partials.  The first collective is issued halfway
        # through phase 1, so the cross-core arrival-skew wait overlaps the
        # rest of phase 1 instead of stalling phase 2.
        half_strips = (nstrip_s + 1) // 2
        splits = [range(0, half_strips), range(half_strips, nstrip_s)]
        rc_lin = dram.tile([npad], F32)
        cp_view = rc_lin[:].rearrange("(j p) -> p j", p=128)
        for gi, strips in enumerate(splits):
            _emit_pass1(
                nc, tc, pools, f1hi_lookup, f2hi_lookup, cp_sb, F_cols, P_rows,
                mybir, strips=strips, exp_inplace=True,
            )
            j0, j1 = strips.start, strips.stop
            cp_local = dram.tile([128, j1 - j0], F32, name=f"cp_local_{gi}")
            cp_red = dram.tile([128, j1 - j0], F32, name=f"cp_red_{gi}")
            nc.sync.dma_start(out=cp_local[:], in_=cp_sb[:, j0:j1])
            nc.gpsimd.collective_compute(
                "AllReduce",
                mybir.AluOpType.add,
                replica_groups=replica_groups,
                ins=[cp_local[:]],
                outs=[cp_red[:]],
            )
            cp_red_sb = consts.tile([128, j1 - j0], F32, name=f"cp_red_sb_{gi}")
            nc.sync.dma_start(out=cp_red_sb[:], in_=cp_red[:])
            rc_small = consts.tile([128, j1 - j0], F32, name=f"rc_small_{gi}")
            nc.vector.reciprocal(out=rc_small[:], in_=cp_red_sb[:])
            nc.sync.dma_start(out=cp_view[:, j0:j1], in_=rc_small[:])
        rc_sb = consts.tile([128, F_cols], F32)
        nc.sync.dma_start(
            out=rc_sb[:], in_=rc_lin[None, :F_cols].to_broadcast((128, F_cols))
        )

        mm_pairs = [
            (f1hi_lookup, f2hi_lookup),
            (f1hi_lookup, f2lo_lookup),
            (f1lo_lookup, f2hi_lookup),
        ]
        _emit_pass2(nc, tc, pools, mm_pairs, rc_sb, conf_out, P_rows, F_cols, mybir)
    nc.compile()
    return nc


def build_colsum_nc(P=S, F=HALF, mm_dtype=PASS1_MM_DTYPE):
    """Two-launch variant, pass 1: per-core partial column sums.
    inputs f1t [128,2,F], f2t [128,2,P]; output colpart [128, nstrip]."""
    import concourse.bacc as bacc
    import concourse.tile as tile
    from concourse import mybir
    from contextlib import ExitStack

    F32 = mybir.dt.float32
    mm_dt = getattr(mybir.dt, mm_dtype)
    nc = bacc.Bacc(None, target_bir_lowering=False)
    f1t = nc.declare_dram_parameter("f1t", [128, 2, F], mm_dt, isOutput=False)
    f2t = nc.declare_dram_parameter("f2t", [128, 2, P], mm_dt, isOutput=False)
    nstrip = (P + 127) // 128
    colpart = nc.declare_dram_parameter("colpart", [128, nstrip], F32, isOutput=True)

    with ExitStack() as ctx:
        tc = ctx.enter_context(tile.TileContext(nc))
        consts = ctx.enter_context(tc.tile_pool(name="consts", bufs=1))
        pools = {
            "psum": ctx.enter_context(tc.tile_pool(name="psum", bufs=2, space="PSUM")),
            "small": ctx.enter_context(tc.tile_pool(name="small", bufs=4)),
        }
        f1t_sb = consts.tile([128, 2, F], mm_dt)
        nc.sync.dma_start(out=f1t_sb[:], in_=f1t[:])
        f2t_lookup = _load_blocked(nc, consts, f2t, P, mm_dt, "f2t")
        cp_sb = consts.tile([128, nstrip], F32)
        nc.vector.memset(cp_sb[:], 0.0)

        def f1t_lookup(k, a, b):
            return f1t_sb[:, k, a:b]

        _emit_pass1(
            nc, tc, pools, f1t_lookup, f2t_lookup, cp_sb, P, F, mybir,
            exp_inplace=True,
        )
        nc.sync.dma_start(out=colpart[:], in_=cp_sb[:])
    nc.compile()
    return nc


def build_conf_nc(P=HALF, F=S, mode=PASS2_MODE):
    """Two-launch variant, pass 2: conf rows for this core's row shard.
    mode "float32": inputs f1t/f2t fp32; "split3": f1hi/f1lo/f2hi/f2lo
    float32r.  Plus rcol [F] = 1/colsum; output conf [P, F]."""
    import concourse.bacc as bacc
    import concourse.tile as tile
    from concourse import mybir
    from contextlib import ExitStack

    F32 = mybir.dt.float32
    nc = bacc.Bacc(None, target_bir_lowering=False)
    names = (
        ["f1hi", "f1lo", "f2hi", "f2lo"] if mode == "split3" else ["f1t", "f2t"]
    )
    mm_dt = mybir.dt.float32r if mode == "split3" else mybir.dt.float32
    params = {}
    for nm in names:
        dim = P if nm.startswith("f1") else F
        params[nm] = nc.declare_dram_parameter(nm, [128, 2, dim], mm_dt, isOutput=False)
    rcol = nc.declare_dram_parameter("rcol", [F], F32, isOutput=False)
    conf_out = nc.declare_dram_parameter("conf", [P, F], F32, isOutput=True)

    with ExitStack() as ctx:
        tc = ctx.enter_context(tile.TileContext(nc))
        consts = ctx.enter_context(tc.tile_pool(name="consts", bufs=1))
        pools = {
            "psum": ctx.enter_context(tc.tile_pool(name="psum", bufs=2, space="PSUM")),
            "strip": ctx.enter_context(tc.tile_pool(name="strip", bufs=2)),
            "small": ctx.enter_context(tc.tile_pool(name="small", bufs=4)),
        }
        lookups = {}
        for nm in names:
            if nm.startswith("f1"):
                t = consts.tile([128, 2, P], mm_dt, name=f"{nm}_sb")
                nc.sync.dma_start(out=t[:], in_=params[nm][:])
                lookups[nm] = (lambda t: lambda k, a, b: t[:, k, a:b])(t)
            else:
                lookups[nm] = _load_blocked(nc, consts, params[nm], F, mm_dt, nm)
        rc_sb = consts.tile([128, F], F32)
        nc.sync.dma_start(out=rc_sb[:], in_=rcol[None, :].to_broadcast((128, F)))

        if mode == "split3":
            mm_pairs = [
                (lookups["f1hi"], lookups["f2hi"]),
                (lookups["f1hi"], lookups["f2lo"]),
                (lookups["f1lo"], lookups["f2hi"]),
            ]
        else:
            mm_pairs = [(lookups["f1t"], lookups["f2t"])]
        _emit_pass2(nc, tc, pools, mm_pairs, rc_sb, conf_out, P, F, mybir)
    nc.compile()
    return nc


def _to_kmajor(x):
    """[Rows, C] f32 -> [128, 2, Rows] with (p, k) = (c % 128, c // 128)."""
    return np.ascontiguousarray(x.T.reshape(2, 128, -1).transpose(1, 0, 2))


_HARDENED = False


def _harden_tracing():
    """Make trace=True (BASS_TRACE=1) survivable in this container: the image's
    antenv lacks axon_hooks (NTFF hook module), and artifact upload has no
    egress. Without this, enabling tracing crashes run_bass_kernel_spmd."""
    global _HARDENED
    if _HARDENED:
        return
    _HARDENED = True
    import types
    import contextlib
    import ctypes

    try:
        import antenv.axon_hooks  # noqa: F401
    except ImportError:
        mod = types.ModuleType("antenv.axon_hooks")
        holder = {"hook": None}
        mod.set_axon_ntff_profile_hook = lambda h: holder.__setitem__("hook", h)
        mod.get_axon_ntff_profile_hook = lambda: holder["hook"]
        try:
            import antenv

            antenv.axon_hooks = mod
        except ImportError:
            pass
        sys.modules["antenv.axon_hooks"] = mod
        try:
            lib = ctypes.CDLL("/opt/axon/libaxon_pjrt.so")
            if hasattr(lib, "axon_start_nrt_profile"):
                lib.axon_start_nrt_profile.argtypes = [
                    ctypes.POINTER(ctypes.c_int64),
                    ctypes.c_size_t,
                ]
                lib.axon_start_nrt_profile.restype = ctypes.c_int64
                lib.axon_stop_nrt_profile.argtypes = [ctypes.c_char_p]
                lib.axon_stop_nrt_profile.restype = ctypes.c_int64

                @contextlib.contextmanager
                def _hook(output_dir, device_ids):
                    import jax

                    jax.devices()
                    if device_ids:
                        ids = (ctypes.c_int64 * len(device_ids))(*device_ids)
                        rc = lib.axon_start_nrt_profile(ids, len(device_ids))
                    else:
                        rc = lib.axon_start_nrt_profile(None, 0)
                    if rc != 0:
                        raise RuntimeError(f"axon_start_nrt_profile rc={rc}")
                    try:
                        yield
                    finally:
                        n = lib.axon_stop_nrt_profile(str(output_dir).encode())
                        print(f"ntff profile: {n} file(s) -> {output_dir}")

                mod.set_axon_ntff_profile_hook(_hook)
        except OSError:
            pass

    from concourse import bass_utils as _bu

    if not getattr(_bu.upload_artifacts, "_is_safe_wrapper", False):
        _orig = _bu.upload_artifacts

        def _safe_upload(tmpdir):
            try:
                return _orig(tmpdir)
            except Exception:
                return str(tmpdir)

        _safe_upload._is_safe_wrapper = True
        _bu.upload_artifacts = _safe_upload


def _host_outputs(conf):
    """Derive the four cheap outputs from conf exactly as the reference."""
    row_max = conf.max(axis=2, keepdims=True)
    col_max = conf.max(axis=1, keepdims=True)
    match_mask = (
        (conf > np.float32(CONFIDENCE_THRESHOLD)) & (conf == row_max) & (conf == col_max)
    )
    column_indices = np.argmax(match_mask, axis=2).astype(np.int32)
    valid = np.any(match_mask, axis=2)
    mc = np.take_along_axis(conf, column_indices[..., None], axis=2)[..., 0]
    matching_confidences = np.where(valid, mc, np.float32(0.0)).astype(np.float32)
    return (matching_confidences, valid, column_indices, match_mask, conf)


def _kernel_fused(f1t_per_core, f2t_per_batch):
    from concourse.bass_utils import run_bass_kernel_spmd

    key = ("fused",)
    if key not in _BUILD_CACHE:
        _BUILD_CACHE[key] = build_fused_nc()
    nc = _BUILD_CACHE[key]

    f1_hl = [split_hi_lo(a) for a in f1t_per_core]
    f2_hl = [split_hi_lo(a) for a in f2t_per_batch]
    in_maps = [
        {
            "f1hi": f1_hl[k][0],
            "f1lo": f1_hl[k][1],
            "f2hi": f2_hl[k // 2][0],
            "f2lo": f2_hl[k // 2][1],
        }
        for k in range(N_CORES)
    ]
    res = run_bass_kernel_spmd(nc, in_maps, core_ids=list(range(N_CORES)))
    LAST_PERF.append(("fused", res.exec_time_ns, res.mean_exec_time_ns))
    conf = np.empty((N, L, S), dtype=np.float32)
    for k in range(N_CORES):
        n, h = k // 2, k % 2
        conf[n, h * HALF : (h + 1) * HALF, :] = res.results[k]["conf"]
    return conf


def _kernel_twopass(f1t_per_core, f2t_per_batch):
    from concourse.bass_utils import run_bass_kernel_spmd

    key = ("twopass", PASS1_MM_DTYPE, PASS2_MODE)
    if key not in _BUILD_CACHE:
        _BUILD_CACHE[key] = (
            build_colsum_nc(mm_dtype=PASS1_MM_DTYPE),
            build_conf_nc(mode=PASS2_MODE),
        )
    nc1, nc2 = _BUILD_CACHE[key]

    if PASS1_MM_DTYPE == "float32r":
        f2t_p1 = [round_mantissa(a) for a in f2t_per_batch]
        f1t_p1 = [round_mantissa(a) for a in f1t_per_core]
    else:
        f2t_p1, f1t_p1 = f2t_per_batch, f1t_per_core
    in_maps1 = [{"f1t": f1t_p1[k], "f2t": f2t_p1[k // 2]} for k in range(N_CORES)]
    res1 = run_bass_kernel_spmd(nc1, in_maps1, core_ids=list(range(N_CORES)))
    LAST_PERF.append(("colsum", res1.exec_time_ns, res1.mean_exec_time_ns))

    # host all-reduce of the column-sum partials (the L-shard reduction)
    colsum = []
    for n in range(N):
        parts = []
        for k in (2 * n, 2 * n + 1):
            a = res1.results[k]["colpart"]  # [128, nstrip]
            parts.append(a.T.reshape(-1)[:S])
        colsum.append(parts[0] + parts[1])
    rcol = [(1.0 / cs.astype(np.float64)).astype(np.float32) for cs in colsum]

    if PASS2_MODE == "split3":
        f1_hl = [split_hi_lo(a) for a in f1t_per_core]
        f2_hl = [split_hi_lo(a) for a in f2t_per_batch]
        in_maps2 = [
            {
                "f1hi": f1_hl[k][0],
                "f1lo": f1_hl[k][1],
                "f2hi": f2_hl[k // 2][0],
                "f2lo": f2_hl[k // 2][1],
                "rcol": rcol[k // 2],
            }
            for k in range(N_CORES)
        ]
    else:
        in_maps2 = [
            {
                "f1t": f1t_per_core[k],
                "f2t": f2t_per_batch[k // 2],
                "rcol": rcol[k // 2],
            }
            for k in range(N_CORES)
        ]
    res2 = run_bass_kernel_spmd(nc2, in_maps2, core_ids=list(range(N_CORES)))
    LAST_PERF.append(("conf", res2.exec_time_ns, res2.mean_exec_time_ns))

    conf = np.empty((N, L, S), dtype=np.float32)
    for k in range(N_CORES):
        n, h = k // 2, k % 2
        conf[n, h * HALF : (h + 1) * HALF, :] = res2.results[k]["conf"]
    return conf


def kernel(coarse_image_feature_1, coarse_image_feature_2):
    _harden_tracing()

    f1 = np.asarray(coarse_image_feature_1, dtype=np.float32)
    f2 = np.asarray(coarse_image_feature_2, dtype=np.float32)
    f1s = f1 * FEAT_SCALE
    f2s = f2 * FEAT_SCALE

    # per-core inputs: core k -> batch k//2, row half k%2
    f2t_per_batch = [_to_kmajor(f2s[n]) for n in range(N)]
    f1t_per_core = [
        _to_kmajor(f1s[k // 2, (k % 2) * HALF : (k % 2 + 1) * HALF])
        for k in range(N_CORES)
    ]

    LAST_PERF.clear()
    if KERNEL_MODE == "fused":
        conf = _kernel_fused(f1t_per_core, f2t_per_batch)
    else:
        conf = _kernel_twopass(f1t_per_core, f2t_per_batch)
    return _host_outputs(conf)


# revision 24
# speedup vs baseline: 1.2785x; 1.2396x over previous
"""Trainium2 Bass kernel for CoarseMatching (dual-softmax feature matching).

Computes, for inputs f1, f2 of shape [N=4, L=4800, C=256]:
    sim  = (f1*s) @ (f2*s)^T / T          (s = C^-0.5, T = 0.1)
    conf = softmax(sim, axis=1) * softmax(sim, axis=2)
plus thresholding / mutual-nearest-neighbour outputs.

Sharding: data-parallel over batch N (4 batches x 2 cores); within a batch
element the L rows are split in half across the 2 cores.  Each core runs a
single fused kernel with two phases.  Phase 1 (transposed orientation)
computes this shard's partial column sums of exp(sim); a pair-wise device
AllReduce (cores {2n, 2n+1}) completes the column-softmax denominator — the
cross-L-shard reduction from the sharding hint.  Phase 2 recomputes exp(sim)
in row orientation and normalizes to conf = E^2/(rowsum*colsum) on device.
The cheap O(L)/bool derived outputs (row/col max, mask, argmax) are formed on
the host from the device conf exactly as the reference does.

Matmul precision strategy: the tensor engine's float32r mode runs at full
rate (1 cyc/row vs 4 for fp32) but consumes only 10 explicit mantissa bits.
Phase 1 runs plain float32r on host-pre-rounded inputs: its output only
feeds 4800-term column sums, where per-element input-rounding noise averages
down to ~1e-5.  Phase 2 (whose matmul error hits conf directly) uses a
3-term hi/lo split (hi.hi + hi.lo + lo.hi, each term exact in float32r)
giving ~2^-21 effective input precision at 3/4 the PE cost of fp32.
"""

import sys

if "/opt/trn_rl_repo" not in sys.path:
    sys.path.insert(0, "/opt/trn_rl_repo")

import os as _os

import numpy as np

N, L, C = 4, 4800, 256
S = L
HALF = L // 2
N_CORES = 8
TEMPERATURE = 0.1
CONFIDENCE_THRESHOLD = 0.2
INV_TEMP = 1.0 / TEMPERATURE  # exp scale applied on device
FEAT_SCALE = np.float32(1.0 / (C**0.5))

# "fused" = single launch with device AllReduce; "twopass" = two launches
# with the 19KB colsum reduction done on host between them.
KERNEL_MODE = _os.environ.get("KERNEL_MODE", "fused")
PASS1_MM_DTYPE = _os.environ.get("KERNEL_P1_DTYPE", "float32r")
# "float32" (exact, 4 cyc/row) or "split3" (3x float32r hi/lo, ~fp32 quality)
PASS2_MODE = _os.environ.get("KERNEL_P2_MODE", "split3")

_BUILD_CACHE = {}

# perf info (exec_time_ns etc.) from the most recent kernel() call, one entry
# per device launch; populated when tracing is enabled (BASS_TRACE=1)
LAST_PERF = []


def _geometry(F):
    """Split free dim F into PSUM blocks (<= 1536 f32 = 3 banks) of matmul
    chunks.  Chunk starts are bank-aligned (512 multiples) and widths are
    256..512 so float32r runs at full rate; block starts are multiples of 128
    so phase-1 partition strips never straddle per-block input tiles."""
    if F == 4800:
        return [
            (0, [512, 512, 512]),
            (1536, [512, 512, 512]),
            (3072, [512, 512, 256]),
            (4352, [448]),
        ]
    if F == 2400:
        return [(0, [512, 512, 512]), (1536, [512, 352])]
    # generic fallback (used by small simulator tests)
    out, f0 = [], 0
    while f0 < F:
        bw = min(1536, F - f0)
        cws, c = [], 0
        while c < bw:
            cw = min(512, bw - c)
            cws.append(cw)
            c += cw
        out.append((f0, cws))
        f0 += bw
    return out


def round_mantissa(x, keep_bits=10):
    """Round fp32 mantissa to keep_bits explicit bits (RNE) — the precision
    the fp32r matmul mode actually consumes; pre-rounding makes it exact."""
    xi = x.view(np.uint32).astype(np.uint64)
    drop = 23 - keep_bits
    half = np.uint64(1 << (drop - 1))
    one = np.uint64(1)
    lsb_mask = np.uint64((1 << drop) - 1)
    rounded = (xi + half - one + ((xi >> np.uint64(drop)) & one)) & ~lsb_mask
    return rounded.astype(np.uint32).view(np.float32)


def split_hi_lo(x, keep_bits=10):
    """x -> (hi, lo): hi = RNE-rounded to keep_bits mantissa bits, lo = the
    exact fp32 residual rounded to keep_bits bits."""
    hi = round_mantissa(x, keep_bits)
    lo = round_mantissa((x - hi).astype(np.float32), keep_bits)
    return hi, lo


class _P:
    """Per-build context bag."""


def _load_blocked(nc, consts, param, F, mm_dt, name):
    """DMA a [128, 2, F] DRAM param into per-geometry-block SBUF tiles so the
    first matmuls only wait for the first block's load.  Returns a lookup
    f(k, a, b) -> AP covering columns [a, b) (must lie inside one block)."""
    blocks = _geometry(F)
    tiles = []
    for f0, cws in blocks:
        bw = sum(cws)
        t = consts.tile([128, 2, bw], mm_dt, name=f"{name}_b{f0}")
        nc.sync.dma_start(out=t[:], in_=param[:, :, f0 : f0 + bw])
        tiles.append((f0, bw, t))

    def lookup(k, a, b):
        for f0, bw, t in tiles:
            if a >= f0 and b <= f0 + bw:
                return t[:, k, a - f0 : b - f0]
        raise AssertionError(f"range [{a},{b}) straddles block tiles")

    return lookup


def _emit_pass1(nc, tc, pools, f1_lookup, f2_lookup, cp_sb, P, F, mybir,
                strips=None, exp_inplace=False):
    """Transposed orientation: for each 128-wide column strip of s, compute
    colpart[s] = sum_l exp(sim[s, l]) over this core's l rows.
    exp_inplace: write the (unused) exp values back onto the PSUM block
    instead of an SBUF scratch tile (ScE->PSUM is the faster port and only
    accum_out matters)."""
    F32 = mybir.dt.float32
    AF = mybir.ActivationFunctionType
    AX = mybir.AxisListType
    psum, small = pools["psum"], pools["small"]
    etmp = pools.get("etmp")
    blocks = _geometry(F)
    nstrip = (P + 127) // 128
    if strips is None:
        strips = range(nstrip)
    for j in strips:
        p0 = j * 128
        pl = min(128, P - p0)
        parts = small.tile([128, len(blocks)], F32, tag="parts1", name=f"parts1_{j}")
        for bi, (f0, cws) in enumerate(blocks):
            bw = sum(cws)
            ps = psum.tile([128, 1536], F32, tag="ps", name=f"ps1_{j}_{bi}")
            c0 = 0
            for cw in cws:
                for k in range(2):
                    nc.tensor.matmul(
                        ps[:pl, c0 : c0 + cw],
                        lhsT=f2_lookup(k, p0, p0 + pl),
                        rhs=f1_lookup(k, f0 + c0, f0 + c0 + cw),
                        start=(k == 0),
                        stop=(k == 1),
                    )
                c0 += cw
            if exp_inplace:
                eout = ps[:pl, :bw]
            else:
                e = etmp.tile([128, 1536], F32, tag="e", name=f"e1_{j}_{bi}")
                eout = e[:pl, :bw]
            nc.scalar.activation(
                out=eout,
                in_=ps[:pl, :bw],
                func=AF.Exp,
                scale=float(INV_TEMP),
                accum_out=parts[:pl, bi : bi + 1],
            )
        nc.vector.reduce_sum(out=cp_sb[:pl, j : j + 1], in_=parts[:pl, :], axis=AX.X)


def _emit_pass2(nc, tc, pools, mm_pair_lookups, rc_sb, conf_out, P, F, mybir):
    """Row orientation: for each 128-row strip of l, compute
    conf = ((E * 1/rowsum) * E) * (1/colsum broadcast) and DMA it out.
    mm_pair_lookups: list of (f1_lookup, f2_lookup) accumulation terms."""
    F32 = mybir.dt.float32
    AF = mybir.ActivationFunctionType
    AX = mybir.AxisListType
    ALU = mybir.AluOpType
    psum, strip_pool, small = pools["psum"], pools["strip"], pools["small"]
    blocks = _geometry(F)
    nstrip = (P + 127) // 128
    nterm = len(mm_pair_lookups)
    for i in range(nstrip):
        p0 = i * 128
        pl = min(128, P - p0)
        parts = small.tile([128, len(blocks)], F32, tag="parts2", name=f"parts2_{i}")
        E = strip_pool.tile([128, F], F32, tag="E", name=f"E_{i}")
        for bi, (f0, cws) in enumerate(blocks):
            bw = sum(cws)
            ps = psum.tile([128, 1536], F32, tag="ps", name=f"ps2_{i}_{bi}")
            c0 = 0
            for cw in cws:
                nmm = 0
                for f1_lookup, f2_lookup in mm_pair_lookups:
                    for k in range(2):
                        nc.tensor.matmul(
                            ps[:pl, c0 : c0 + cw],
                            lhsT=f1_lookup(k, p0, p0 + pl),
                            rhs=f2_lookup(k, f0 + c0, f0 + c0 + cw),
                            start=(nmm == 0),
                            stop=(nmm == 2 * nterm - 1),
                        )
                        nmm += 1
                c0 += cw
            nc.scalar.activation(
                out=E[:pl, f0 : f0 + bw],
                in_=ps[:pl, :bw],
                func=AF.Exp,
                scale=float(INV_TEMP),
                accum_out=parts[:pl, bi : bi + 1],
            )
        rowsum = small.tile([128, 1], F32, tag="rowsum", name=f"rowsum_{i}")
        rr = small.tile([128, 1], F32, tag="rr", name=f"rr_{i}")
        nc.vector.reduce_sum(out=rowsum[:pl], in_=parts[:pl, :], axis=AX.X)
        nc.vector.reciprocal(out=rr[:pl], in_=rowsum[:pl])
        # conf in place in E (SBUF budget: no room for a second strip tile
        # alongside the four hi/lo operand tensors)
        nc.vector.scalar_tensor_tensor(
            out=E[:pl],
            in0=E[:pl],
            scalar=rr[:pl],
            in1=E[:pl],
            op0=ALU.mult,
            op1=ALU.mult,
        )
        # column-scale multiply stays on DVE: GPSIMD shares an SBUF port
        # pair with DVE and offloading there halves DVE throughput
        nc.vector.tensor_tensor(out=E[:pl], in0=E[:pl], in1=rc_sb[:pl], op=ALU.mult)
        nc.sync.dma_start(out=conf_out[p0 : p0 + pl, :], in_=E[:pl, :])


def build_fused_nc(P_rows=HALF, F_cols=S, replica_groups=None):
    """Single-launch fused kernel: phase 1 colsum partials, pair AllReduce,
    reciprocal + broadcast, phase 2 conf.  Inputs: f1hi/f1lo [128,2,P_rows],
    f2hi/f2lo [128,2,F_cols] (float32r hi/lo split); output conf [P_rows,
    F_cols] f32."""
    import concourse.bacc as bacc
    import concourse.tile as tile
    from concourse import mybir
    from contextlib import ExitStack

    F32 = mybir.dt.float32
    mm_dt = mybir.dt.float32r
    if replica_groups is None:
        replica_groups = [[0, 1], [2, 3], [4, 5], [6, 7]]

    nc = bacc.Bacc(None, target_bir_lowering=False, num_devices=N_CORES)
    f1hi = nc.declare_dram_parameter("f1hi", [128, 2, P_rows], mm_dt, isOutput=False)
    f1lo = nc.declare_dram_parameter("f1lo", [128, 2, P_rows], mm_dt, isOutput=False)
    f2hi = nc.declare_dram_parameter("f2hi", [128, 2, F_cols], mm_dt, isOutput=False)
    f2lo = nc.declare_dram_parameter("f2lo", [128, 2, F_cols], mm_dt, isOutput=False)
    conf_out = nc.declare_dram_parameter("conf", [P_rows, F_cols], F32, isOutput=True)
    nstrip_s = (F_cols + 127) // 128
    npad = nstrip_s * 128

    with ExitStack() as ctx:
        tc = ctx.enter_context(tile.TileContext(nc))
        consts = ctx.enter_context(tc.tile_pool(name="consts", bufs=1))
        pools = {
            "psum": ctx.enter_context(tc.tile_pool(name="psum", bufs=2, space="PSUM")),
            "strip": ctx.enter_context(tc.tile_pool(name="strip", bufs=3)),
            "small": ctx.enter_context(tc.tile_pool(name="small", bufs=4)),
        }
        dram = ctx.enter_context(tc.tile_pool(name="dram", bufs=1, space="DRAM"))

        # phase-1-critical loads first: f1hi (rhs, full rows) + f2hi blocks
        # (lhsT strips); lo tensors only gate phase 2.
        f1hi_sb = consts.tile([128, 2, P_rows], mm_dt)
        nc.sync.dma_start(out=f1hi_sb[:], in_=f1hi[:])
        f2hi_lookup = _load_blocked(nc, consts, f2hi, F_cols, mm_dt, "f2hi")
        f1lo_sb = consts.tile([128, 2, P_rows], mm_dt)
        nc.sync.dma_start(out=f1lo_sb[:], in_=f1lo[:])
        f2lo_lookup = _load_blocked(nc, consts, f2lo, F_cols, mm_dt, "f2lo")

        def f1hi_lookup(k, a, b):
            return f1hi_sb[:, k, a:b]

        def f1lo_lookup(k, a, b):
            return f1lo_sb[:, k, a:b]

        cp_sb = consts.tile([128, nstrip_s], F32)
        nc.vector.memset(cp_sb[:], 0.0)

        # phase 1 in four chunks, each followed by its own pair-AllReduce of
        # the colsum partials.  The first collective is issued a quarter of
        # the way through phase 1, so the cross-core arrival-skew wait
        # overlaps the rest of phase 1; the last chunk's reduce is small and
        # completes shortly after phase 1 ends.
        nchunk = 4 if nstrip_s >= 4 else 2
        step = (nstrip_s + nchunk - 1) // nchunk
        splits = [
            range(a, min(a + step, nstrip_s)) for a in range(0, nstrip_s, step)
        ]
        rc_lin = dram.tile([npad], F32)
        cp_view = rc_lin[:].rearrange("(j p) -> p j", p=128)
        for gi, strips in enumerate(splits):
            _emit_pass1(
                nc, tc, pools, f1hi_lookup, f2hi_lookup, cp_sb, F_cols, P_rows,
                mybir, strips=strips, exp_inplace=True,
            )
            j0, j1 = strips.start, strips.stop
            cp_local = dram.tile([128, j1 - j0], F32, name=f"cp_local_{gi}")
            cp_red = dram.tile([128, j1 - j0], F32, name=f"cp_red_{gi}")
            nc.sync.dma_start(out=cp_local[:], in_=cp_sb[:, j0:j1])
            nc.gpsimd.collective_compute(
                "AllReduce",
                mybir.AluOpType.add,
                replica_groups=replica_groups,
                ins=[cp_local[:]],
                outs=[cp_red[:]],
            )
            cp_red_sb = consts.tile([128, j1 - j0], F32, name=f"cp_red_sb_{gi}")
            nc.sync.dma_start(out=cp_red_sb[:], in_=cp_red[:])
            rc_small = consts.tile([128, j1 - j0], F32, name=f"rc_small_{gi}")
            nc.vector.reciprocal(out=rc_small[:], in_=cp_red_sb[:])
            nc.sync.dma_start(out=cp_view[:, j0:j1], in_=rc_small[:])
        rc_sb = consts.tile([128, F_cols], F32)
        nc.sync.dma_start(
            out=rc_sb[:], in_=rc_lin[None, :F_cols].to_broadcast((128, F_cols))
        )

        mm_pairs = [
            (f1hi_lookup, f2hi_lookup),
            (f1hi_lookup, f2lo_lookup),
            (f1lo_lookup, f2hi_lookup),
        ]
        _emit_pass2(nc, tc, pools, mm_pairs, rc_sb, conf_out, P_rows, F_cols, mybir)
    nc.compile()
    return nc


def build_colsum_nc(P=S, F=HALF, mm_dtype=PASS1_MM_DTYPE):
    """Two-launch variant, pass 1: per-core partial column sums.
    inputs f1t [128,2,F], f2t [128,2,P]; output colpart [128, nstrip]."""
    import concourse.bacc as bacc
    import concourse.tile as tile
    from concourse import mybir
    from contextlib import ExitStack

    F32 = mybir.dt.float32
    mm_dt = getattr(mybir.dt, mm_dtype)
    nc = bacc.Bacc(None, target_bir_lowering=False)
    f1t = nc.declare_dram_parameter("f1t", [128, 2, F], mm_dt, isOutput=False)
    f2t = nc.declare_dram_parameter("f2t", [128, 2, P], mm_dt, isOutput=False)
    nstrip = (P + 127) // 128
    colpart = nc.declare_dram_parameter("colpart", [128, nstrip], F32, isOutput=True)

    with ExitStack() as ctx:
        tc = ctx.enter_context(tile.TileContext(nc))
        consts = ctx.enter_context(tc.tile_pool(name="consts", bufs=1))
        pools = {
            "psum": ctx.enter_context(tc.tile_pool(name="psum", bufs=2, space="PSUM")),
            "small": ctx.enter_context(tc.tile_pool(name="small", bufs=4)),
        }
        f1t_sb = consts.tile([128, 2, F], mm_dt)
        nc.sync.dma_start(out=f1t_sb[:], in_=f1t[:])
        f2t_lookup = _load_blocked(nc, consts, f2t, P, mm_dt, "f2t")
        cp_sb = consts.tile([128, nstrip], F32)
        nc.vector.memset(cp_sb[:], 0.0)

        def f1t_lookup(k, a, b):
            return f1t_sb[:, k, a:b]

        _emit_pass1(
            nc, tc, pools, f1t_lookup, f2t_lookup, cp_sb, P, F, mybir,
            exp_inplace=True,
        )
        nc.sync.dma_start(out=colpart[:], in_=cp_sb[:])
    nc.compile()
    return nc


def build_conf_nc(P=HALF, F=S, mode=PASS2_MODE):
    """Two-launch variant, pass 2: conf rows for this core's row shard.
    mode "float32": inputs f1t/f2t fp32; "split3": f1hi/f1lo/f2hi/f2lo
    float32r.  Plus rcol [F] = 1/colsum; output conf [P, F]."""
    import concourse.bacc as bacc
    import concourse.tile as tile
    from concourse import mybir
    from contextlib import ExitStack

    F32 = mybir.dt.float32
    nc = bacc.Bacc(None, target_bir_lowering=False)
    names = (
        ["f1hi", "f1lo", "f2hi", "f2lo"] if mode == "split3" else ["f1t", "f2t"]
    )
    mm_dt = mybir.dt.float32r if mode == "split3" else mybir.dt.float32
    params = {}
    for nm in names:
        dim = P if nm.startswith("f1") else F
        params[nm] = nc.declare_dram_parameter(nm, [128, 2, dim], mm_dt, isOutput=False)
    rcol = nc.declare_dram_parameter("rcol", [F], F32, isOutput=False)
    conf_out = nc.declare_dram_parameter("conf", [P, F], F32, isOutput=True)

    with ExitStack() as ctx:
        tc = ctx.enter_context(tile.TileContext(nc))
        consts = ctx.enter_context(tc.tile_pool(name="consts", bufs=1))
        pools = {
            "psum": ctx.enter_context(tc.tile_pool(name="psum", bufs=2, space="PSUM")),
            "strip": ctx.enter_context(tc.tile_pool(name="strip", bufs=2)),
            "small": ctx.enter_context(tc.tile_pool(name="small", bufs=4)),
        }
        lookups = {}
        for nm in names:
            if nm.startswith("f1"):
                t = consts.tile([128, 2, P], mm_dt, name=f"{nm}_sb")
                nc.sync.dma_start(out=t[:], in_=params[nm][:])
                lookups[nm] = (lambda t: lambda k, a, b: t[:, k, a:b])(t)
            else:
                lookups[nm] = _load_blocked(nc, consts, params[nm], F, mm_dt, nm)
        rc_sb = consts.tile([128, F], F32)
        nc.sync.dma_start(out=rc_sb[:], in_=rcol[None, :].to_broadcast((128, F)))

        if mode == "split3":
            mm_pairs = [
                (lookups["f1hi"], lookups["f2hi"]),
                (lookups["f1hi"], lookups["f2lo"]),
                (lookups["f1lo"], lookups["f2hi"]),
            ]
        else:
            mm_pairs = [(lookups["f1t"], lookups["f2t"])]
        _emit_pass2(nc, tc, pools, mm_pairs, rc_sb, conf_out, P, F, mybir)
    nc.compile()
    return nc


def _to_kmajor(x):
    """[Rows, C] f32 -> [128, 2, Rows] with (p, k) = (c % 128, c // 128)."""
    return np.ascontiguousarray(x.T.reshape(2, 128, -1).transpose(1, 0, 2))


_HARDENED = False


def _harden_tracing():
    """Make trace=True (BASS_TRACE=1) survivable in this container: the image's
    antenv lacks axon_hooks (NTFF hook module), and artifact upload has no
    egress. Without this, enabling tracing crashes run_bass_kernel_spmd."""
    global _HARDENED
    if _HARDENED:
        return
    _HARDENED = True
    import types
    import contextlib
    import ctypes

    try:
        import antenv.axon_hooks  # noqa: F401
    except ImportError:
        mod = types.ModuleType("antenv.axon_hooks")
        holder = {"hook": None}
        mod.set_axon_ntff_profile_hook = lambda h: holder.__setitem__("hook", h)
        mod.get_axon_ntff_profile_hook = lambda: holder["hook"]
        try:
            import antenv

            antenv.axon_hooks = mod
        except ImportError:
            pass
        sys.modules["antenv.axon_hooks"] = mod
        try:
            lib = ctypes.CDLL("/opt/axon/libaxon_pjrt.so")
            if hasattr(lib, "axon_start_nrt_profile"):
                lib.axon_start_nrt_profile.argtypes = [
                    ctypes.POINTER(ctypes.c_int64),
                    ctypes.c_size_t,
                ]
                lib.axon_start_nrt_profile.restype = ctypes.c_int64
                lib.axon_stop_nrt_profile.argtypes = [ctypes.c_char_p]
                lib.axon_stop_nrt_profile.restype = ctypes.c_int64

                @contextlib.contextmanager
                def _hook(output_dir, device_ids):
                    import jax

                    jax.devices()
                    if device_ids:
                        ids = (ctypes.c_int64 * len(device_ids))(*device_ids)
                        rc = lib.axon_start_nrt_profile(ids, len(device_ids))
                    else:
                        rc = lib.axon_start_nrt_profile(None, 0)
                    if rc != 0:
                        raise RuntimeError(f"axon_start_nrt_profile rc={rc}")
                    try:
                        yield
                    finally:
                        n = lib.axon_stop_nrt_profile(str(output_dir).encode())
                        print(f"ntff profile: {n} file(s) -> {output_dir}")

                mod.set_axon_ntff_profile_hook(_hook)
        except OSError:
            pass

    from concourse import bass_utils as _bu

    if not getattr(_bu.upload_artifacts, "_is_safe_wrapper", False):
        _orig = _bu.upload_artifacts

        def _safe_upload(tmpdir):
            try:
                return _orig(tmpdir)
            except Exception:
                return str(tmpdir)

        _safe_upload._is_safe_wrapper = True
        _bu.upload_artifacts = _safe_upload


def _host_outputs(conf):
    """Derive the four cheap outputs from conf exactly as the reference."""
    row_max = conf.max(axis=2, keepdims=True)
    col_max = conf.max(axis=1, keepdims=True)
    match_mask = (
        (conf > np.float32(CONFIDENCE_THRESHOLD)) & (conf == row_max) & (conf == col_max)
    )
    column_indices = np.argmax(match_mask, axis=2).astype(np.int32)
    valid = np.any(match_mask, axis=2)
    mc = np.take_along_axis(conf, column_indices[..., None], axis=2)[..., 0]
    matching_confidences = np.where(valid, mc, np.float32(0.0)).astype(np.float32)
    return (matching_confidences, valid, column_indices, match_mask, conf)


def _kernel_fused(f1t_per_core, f2t_per_batch):
    from concourse.bass_utils import run_bass_kernel_spmd

    key = ("fused",)
    if key not in _BUILD_CACHE:
        _BUILD_CACHE[key] = build_fused_nc()
    nc = _BUILD_CACHE[key]

    f1_hl = [split_hi_lo(a) for a in f1t_per_core]
    f2_hl = [split_hi_lo(a) for a in f2t_per_batch]
    in_maps = [
        {
            "f1hi": f1_hl[k][0],
            "f1lo": f1_hl[k][1],
            "f2hi": f2_hl[k // 2][0],
            "f2lo": f2_hl[k // 2][1],
        }
        for k in range(N_CORES)
    ]
    res = run_bass_kernel_spmd(nc, in_maps, core_ids=list(range(N_CORES)))
    LAST_PERF.append(("fused", res.exec_time_ns, res.mean_exec_time_ns))
    conf = np.empty((N, L, S), dtype=np.float32)
    for k in range(N_CORES):
        n, h = k // 2, k % 2
        conf[n, h * HALF : (h + 1) * HALF, :] = res.results[k]["conf"]
    return conf


def _kernel_twopass(f1t_per_core, f2t_per_batch):
    from concourse.bass_utils import run_bass_kernel_spmd

    key = ("twopass", PASS1_MM_DTYPE, PASS2_MODE)
    if key not in _BUILD_CACHE:
        _BUILD_CACHE[key] = (
            build_colsum_nc(mm_dtype=PASS1_MM_DTYPE),
            build_conf_nc(mode=PASS2_MODE),
        )
    nc1, nc2 = _BUILD_CACHE[key]

    if PASS1_MM_DTYPE == "float32r":
        f2t_p1 = [round_mantissa(a) for a in f2t_per_batch]
        f1t_p1 = [round_mantissa(a) for a in f1t_per_core]
    else:
        f2t_p1, f1t_p1 = f2t_per_batch, f1t_per_core
    in_maps1 = [{"f1t": f1t_p1[k], "f2t": f2t_p1[k // 2]} for k in range(N_CORES)]
    res1 = run_bass_kernel_spmd(nc1, in_maps1, core_ids=list(range(N_CORES)))
    LAST_PERF.append(("colsum", res1.exec_time_ns, res1.mean_exec_time_ns))

    # host all-reduce of the column-sum partials (the L-shard reduction)
    colsum = []
    for n in range(N):
        parts = []
        for k in (2 * n, 2 * n + 1):
            a = res1.results[k]["colpart"]  # [128, nstrip]
            parts.append(a.T.reshape(-1)[:S])
        colsum.append(parts[0] + parts[1])
    rcol = [(1.0 / cs.astype(np.float64)).astype(np.float32) for cs in colsum]

    if PASS2_MODE == "split3":
        f1_hl = [split_hi_lo(a) for a in f1t_per_core]
        f2_hl = [split_hi_lo(a) for a in f2t_per_batch]
        in_maps2 = [
            {
                "f1hi": f1_hl[k][0],
                "f1lo": f1_hl[k][1],
                "f2hi": f2_hl[k // 2][0],
                "f2lo": f2_hl[k // 2][1],
                "rcol": rcol[k // 2],
            }
            for k in range(N_CORES)
        ]
    else:
        in_maps2 = [
            {
                "f1t": f1t_per_core[k],
                "f2t": f2t_per_batch[k // 2],
                "rcol": rcol[k // 2],
            }
            for k in range(N_CORES)
        ]
    res2 = run_bass_kernel_spmd(nc2, in_maps2, core_ids=list(range(N_CORES)))
    LAST_PERF.append(("conf", res2.exec_time_ns, res2.mean_exec_time_ns))

    conf = np.empty((N, L, S), dtype=np.float32)
    for k in range(N_CORES):
        n, h = k // 2, k % 2
        conf[n, h * HALF : (h + 1) * HALF, :] = res2.results[k]["conf"]
    return conf


def kernel(coarse_image_feature_1, coarse_image_feature_2):
    _harden_tracing()

    f1 = np.asarray(coarse_image_feature_1, dtype=np.float32)
    f2 = np.asarray(coarse_image_feature_2, dtype=np.float32)
    f1s = f1 * FEAT_SCALE
    f2s = f2 * FEAT_SCALE

    # per-core inputs: core k -> batch k//2, row half k%2
    f2t_per_batch = [_to_kmajor(f2s[n]) for n in range(N)]
    f1t_per_core = [
        _to_kmajor(f1s[k // 2, (k % 2) * HALF : (k % 2 + 1) * HALF])
        for k in range(N_CORES)
    ]

    LAST_PERF.clear()
    if KERNEL_MODE == "fused":
        conf = _kernel_fused(f1t_per_core, f2t_per_batch)
    else:
        conf = _kernel_twopass(f1t_per_core, f2t_per_batch)
    return _host_outputs(conf)
